# revision 5
# baseline (speedup 1.0000x reference)
"""Multi-head attention Trainium2 kernel (B=8, N=1024, C=768, H=12, d=64).

Sharding: data-parallel over batch -- core b computes batch element b.

Per-core dataflow (fp16 matmul operands, fp32 PSUM accumulation; fp16 keeps
the PE on its full-clock datapath):
  - host pre-transposes x -> xT [C, N] and all weights -> [in, out] layout,
    folds the 1/sqrt(d) softmax scale into q_w, extends v_w with a zero
    column per head (slot for the softmax-denominator ones trick).
  - Qt = wqT.T @ xT   [C, N]  (transposed layout, heads on partitions)
  - Kt = wkT.T @ xT   [C, N]
  - V' = xT.T @ vwT'  [N, H*65]  (natural layout; col h*65+64 memset to 1.0)
  - per head pair t, token-chunk ch: both heads' transposed scores land in
    one 2-bank PSUM tile st[128, 2, 512]; ONE Exp activation covers the
    pair. The P@V' accumulation runs one m-tile BEHIND the score stream so
    the PE never waits on the just-issued Exp (software pipeline).
    yt'[d'|sum, n] = V'_h.T @ P accumulated over m-tiles; row 64 = colsum
  - per (t, ch): Yt = yt * head_mask[h]^2 / colsum, normalized immediately
    (reciprocal_approx_fast straight off the PSUM colsum rows; partition
    broadcast on GpSimd) so the tail only waits on the final chunk.
  - out = Yt.T @ pwT  [N, C], staged fp16 (ACT-engine eviction), host casts
    back to fp32.
"""

import numpy as np

B, N, C, H, D = 8, 1024, 768, 12, 64
KO = C // 128          # 6 contraction tiles of 128 channels
MT = N // 128          # 8 token tiles
NCH = N // 512         # 2 free-dim chunks of 512
D1 = D + 1             # V' block width per head (64 V cols + 1 ones col)
CV = H * D1            # 780 extended V channels
NCORES = 8

MM_DTYPE = "f16"

_cache = {}


def _build():
    import concourse.bacc as bacc
    import concourse.mybir as mybir
    import concourse.tile as tile

    F32 = mybir.dt.float32
    MMD = {"bf16": mybir.dt.bfloat16, "f16": mybir.dt.float16,
           "f32r": mybir.dt.float32r, "f32": mybir.dt.float32}[MM_DTYPE]
    AF = mybir.ActivationFunctionType

    nc = bacc.Bacc("TRN2", target_bir_lowering=False, debug=False)

    d_xT = nc.dram_tensor("xT", [C, N], MMD, kind="ExternalInput")
    d_wq = nc.dram_tensor("wqT", [C, C], MMD, kind="ExternalInput")
    d_wk = nc.dram_tensor("wkT", [C, C], MMD, kind="ExternalInput")
    d_wv = nc.dram_tensor("vwT", [C, CV], MMD, kind="ExternalInput")
    d_wp = nc.dram_tensor("pwT", [C, C], MMD, kind="ExternalInput")
    d_out = nc.dram_tensor("out", [N, C], MMD, kind="ExternalOutput")

    r_xT = d_xT.ap().rearrange("(ko p) n -> p ko n", p=128)
    r_wq = d_wq.ap().rearrange("(ko p) m -> p ko m", p=128)
    r_wk = d_wk.ap().rearrange("(ko p) m -> p ko m", p=128)
    r_wv = d_wv.ap().rearrange("(ko p) m -> p ko m", p=128)
    r_wp = d_wp.ap().rearrange("(ko p) m -> p ko m", p=128)
    r_out = d_out.ap().rearrange("(mt p) c -> mt p c", p=128)

    with tile.TileContext(nc) as tc:
        with (
            tc.tile_pool(name="xw", bufs=1) as xw,          # xT, vwT, wp (resident)
            tc.tile_pool(name="wq", bufs=3) as wqp,         # streamed weight blocks
            tc.tile_pool(name="wk", bufs=3) as wkp,
            tc.tile_pool(name="qt", bufs=3) as qtp,         # Qt/Kt streamed per pair
            tc.tile_pool(name="kt", bufs=3) as ktp,
            tc.tile_pool(name="vp", bufs=8) as vpp,         # V' all 8 token tiles
            tc.tile_pool(name="yt", bufs=6) as ytp,         # Yt all 6 channel tiles
            tc.tile_pool(name="pp", bufs=6) as ppp,         # P = exp(St), paired
            tc.tile_pool(name="cs", bufs=4) as csp,         # recip rows
            tc.tile_pool(name="bc", bufs=3) as bcp,         # broadcast tiles
            tc.tile_pool(name="ob", bufs=2) as obp,         # output staging
            tc.tile_pool(name="mm", bufs=2, space="PSUM") as mmp,
            tc.tile_pool(name="st", bufs=2, space="PSUM") as stp,
            tc.tile_pool(name="ya", bufs=2, space="PSUM") as yap,
        ):
            # ---- resident tiles ----
            t_x = xw.tile([128, KO, N], MMD, tag="x")
            t_wv = xw.tile([128, KO, CV], MMD, tag="wv")
            t_wp = xw.tile([128, KO, C], MMD, tag="wpf")

            def make_qk(t, dma_engine):
                """DMA the weight blocks for channel tile t and return
                (t_q, t_k, units) where units are deferred emitters, each
                HALF a PSUM accumulation group (3 matmuls; 2nd half also
                evicts)."""
                t_wqb = wqp.tile([128, KO, 128], MMD, tag="wq", name=f"wqb{t}")
                dma_engine.dma_start(
                    out=t_wqb[:], in_=r_wq[:, :, t * 128:(t + 1) * 128]
                )
                t_wkb = wkp.tile([128, KO, 128], MMD, tag="wk", name=f"wkb{t}")
                dma_engine.dma_start(
                    out=t_wkb[:], in_=r_wk[:, :, t * 128:(t + 1) * 128]
                )
                t_q = qtp.tile([128, N], MMD, tag="qt", name=f"q{t}")
                t_k = ktp.tile([128, N], MMD, tag="kt", name=f"k{t}")

                def unit(wsrc, dst, ch, nm):
                    nsl = slice(ch * 512, (ch + 1) * 512)
                    state = {}

                    def part_a():
                        ps = mmp.tile([128, 512], F32, tag="mm", name=nm)
                        state["ps"] = ps
                        for ko in range(3):
                            nc.tensor.matmul(
                                ps[:], wsrc[:, ko, :], t_x[:, ko, nsl],
                                start=(ko == 0), stop=False,
                            )

                    def part_b():
                        ps = state["ps"]
                        for ko in range(3, KO):
                            nc.tensor.matmul(
                                ps[:], wsrc[:, ko, :], t_x[:, ko, nsl],
                                start=False, stop=(ko == KO - 1),
                            )
                        nc.vector.tensor_copy(dst[:, nsl], ps[:])

                    return [part_a, part_b]

                units = []
                units += unit(t_wqb, t_q, 0, f"pq{t}a")
                units += unit(t_wkb, t_k, 0, f"pk{t}a")
                units += unit(t_wqb, t_q, 1, f"pq{t}b")
                units += unit(t_wkb, t_k, 1, f"pk{t}b")
                return t_q, t_k, units

            # ---- head DMAs, spread across idle engine queues so configs
            # issue in parallel: first-needed operands first ----
            t_q, t_k, units0 = make_qk(0, nc.sync)
            # x: 4 chunks keyed to consumption order (QK proj ch0 ko-halves
            # first); scalar + vector engines are idle here
            nc.scalar.dma_start(out=t_x[:, 0:3, 0:512], in_=r_xT[:, 0:3, 0:512])
            nc.sync.dma_start(out=t_x[:, 3:6, 0:512], in_=r_xT[:, 3:6, 0:512])
            nc.scalar.dma_start(out=t_x[:, 0:3, 512:1024],
                                in_=r_xT[:, 0:3, 512:1024])
            nc.sync.dma_start(out=t_x[:, 3:6, 512:1024],
                              in_=r_xT[:, 3:6, 512:1024])
            nc.gpsimd.dma_start(out=t_wv[:, 0:3, :], in_=r_wv[:, 0:3, :])
            nc.gpsimd.dma_start(out=t_wv[:, 3:6, :], in_=r_wv[:, 3:6, :])
            nc.gpsimd.dma_start(out=t_wp[:], in_=r_wp[:])

            for u in units0:
                u()
            qk_tiles = {0: (t_q, t_k)}
            created = 0
            pend = []  # (need_by_tile_idx, deferred emitter)

            # ---- V' projection units (weavable): V'[n, cv] = xT.T @ vwT ----
            t_v = [vpp.tile([128, CV], MMD, tag="v", name=f"v{mt}")
                   for mt in range(MT)]
            vch = [(0, 390), (390, 390)]

            def v_unit(mt):
                tv = t_v[mt]
                state = {}

                def part_a():
                    ps = mmp.tile([128, 512], F32, tag="mm", name=f"v{mt}a")
                    state["ps"] = ps
                    for ko in range(KO):
                        nc.tensor.matmul(
                            ps[:, :390], t_x[:, ko, mt * 128:(mt + 1) * 128],
                            t_wv[:, ko, 0:390],
                            start=(ko == 0), stop=(ko == KO - 1),
                        )
                    nc.vector.tensor_copy(tv[:, 0:390], ps[:, :390])

                def part_b():
                    ps = mmp.tile([128, 512], F32, tag="mm", name=f"v{mt}b")
                    for ko in range(KO):
                        nc.tensor.matmul(
                            ps[:, :390], t_x[:, ko, mt * 128:(mt + 1) * 128],
                            t_wv[:, ko, 390:780],
                            start=(ko == 0), stop=(ko == KO - 1),
                        )
                    nc.vector.tensor_copy(tv[:, 390:780], ps[:, :390])
                    ones_cols = tv[:].rearrange(
                        "p (h e) -> p h e", e=D1)[:, :, D:D + 1]
                    nc.vector.memset(ones_cols, 1.0)

                return [part_a, part_b]

            # V'[0] and V'[1] up front (needed by the first PV steps);
            # the rest weave into pair 0's attention stream
            for mt in range(2):
                for u in v_unit(mt):
                    u()
            for mt in range(2, MT):
                pend.extend((0, u) for u in v_unit(mt))

            t_yt = [ytp.tile([128, N], MMD, tag="yt", name=f"yt{i}")
                    for i in range(KO)]

            # ---- per channel-tile: attention pair with PV software-pipelined
            # one m-tile behind the score stream; projection half-groups for
            # later pairs woven in as PE filler ----

            for t in range(KO):
                while created < min(t + 2, KO - 1):
                    created += 1
                    q_, k_, us = make_qk(created, nc.gpsimd)
                    qk_tiles[created] = (q_, k_)
                    pend.extend((created, u) for u in us)

                for ch in range(NCH):
                    nsl = slice(ch * 512, (ch + 1) * 512)
                    yt0 = yap.tile([D1, 512], F32, tag="ya", name=f"ya{t}{ch}0")
                    yt1 = yap.tile([D1, 512], F32, tag="ya", name=f"ya{t}{ch}1")
                    p_tiles = [None] * MT
                    for step in range(MT + 1):
                        if step < MT:
                            mt = step
                            msl = slice(mt * 128, (mt + 1) * 128)
                            st = stp.tile([128, 2, 512], F32, tag="st",
                                          name=f"st{t}{ch}{mt}")
                            nc.tensor.matmul(
                                st[:, 0, :], t_k[0:64, msl], t_q[0:64, nsl],
                                start=True, stop=True, tile_position=(0, 0),
                            )
                            nc.tensor.matmul(
                                st[:, 1, :], t_k[64:128, msl], t_q[64:128, nsl],
                                start=True, stop=True, tile_position=(64, 0),
                            )
                            p = ppp.tile([128, 2, 512], MMD, tag="p",
                                         name=f"p{t}{ch}{mt}")
                            nc.scalar.activation(p[:], st[:], AF.Exp)
                            p_tiles[mt] = p
                        if step > 0:
                            mt = step - 1
                            p = p_tiles[mt]
                            nc.tensor.matmul(
                                yt0[:], t_v[mt][:, (2 * t) * D1:(2 * t + 1) * D1],
                                p[:, 0, :], start=(mt == 0), stop=(mt == MT - 1),
                            )
                            nc.tensor.matmul(
                                yt1[:],
                                t_v[mt][:, (2 * t + 1) * D1:(2 * t + 2) * D1],
                                p[:, 1, :], start=(mt == 0), stop=(mt == MT - 1),
                            )
                        # weave projection half-groups into the stream
                        # (double rate during pair 0 to finish V' in time)
                        npop = 2 if t == 0 else 1
                        for _ in range(npop):
                            if pend:
                                pend.pop(0)[1]()
                    # reciprocals straight off the PSUM colsum rows, then
                    # evict + normalize this chunk on DVE/GpSimd while the
                    # PE streams on
                    t_cs = csp.tile([1, 2, 512], F32, tag="cs", name=f"cs{t}{ch}")
                    nc.vector.tensor_copy(t_cs[0:1, 0, :], yt0[D:D1, :])
                    nc.vector.tensor_copy(t_cs[0:1, 1, :], yt1[D:D1, :])
                    t_rc = csp.tile([1, 2, 512], F32, tag="rc", name=f"rc{t}{ch}")
                    nc.vector.reciprocal_approx_fast(t_rc[:], t_cs[:])
                    nc.vector.tensor_copy(t_yt[t][0:64, nsl], yt0[0:D, :])
                    nc.vector.tensor_copy(t_yt[t][64:128, nsl], yt1[0:D, :])
                    for hp in range(2):
                        psl = slice(hp * 64, hp * 64 + 64)
                        t_bc = bcp.tile([128, 512], F32, tag="bc",
                                        name=f"bc{t}{ch}{hp}")
                        nc.gpsimd.partition_broadcast(
                            t_bc[:], t_rc[0:1, hp, :]
                        )
                        nc.vector.tensor_mul(
                            t_yt[t][psl, nsl], t_yt[t][psl, nsl], t_bc[psl, :]
                        )
                # next pair's projections must be complete before it starts
                for i, u in [pu for pu in pend if pu[0] == t + 1]:
                    u()
                pend = [pu for pu in pend if pu[0] != t + 1]
                if t + 1 < KO:
                    t_q, t_k = qk_tiles[t + 1]

            # ---- output projection: out[n, c] = Yt.T @ pwT; yt stationary
            # shared by both output chunks (halves the LDWEIGHTS); eviction
            # on the ACT engine (Copy shares the Exp table) ----
            for mt in range(MT):
                t_o = obp.tile([128, C], MMD, tag="ob", name=f"ob{mt}")
                msl = slice(mt * 128, (mt + 1) * 128)
                psA = mmp.tile([128, 512], F32, tag="mm", name=f"poA{mt}")
                psB = mmp.tile([128, 512], F32, tag="mm", name=f"poB{mt}")
                for t in range(KO):
                    nc.tensor.matmul(
                        psA[:], t_yt[t][:, msl], t_wp[:, t, 0:512],
                        start=(t == 0), stop=(t == KO - 1),
                    )
                    nc.tensor.matmul(
                        psB[:, 0:256], t_yt[t][:, msl], t_wp[:, t, 512:768],
                        start=(t == 0), stop=(t == KO - 1),
                    )
                nc.scalar.copy(t_o[:, 0:512], psA[:])
                nc.scalar.copy(t_o[:, 512:768], psB[:, 0:256])
                nc.sync.dma_start(out=r_out[mt, :, :], in_=t_o[:])

    nc.compile()
    return nc


def _prep_inputs(x, head_mask, q_w, k_w, v_w, proj_w):
    import ml_dtypes

    mmnp = {"bf16": ml_dtypes.bfloat16, "f16": np.float16,
            "f32r": np.float32, "f32": np.float32}[MM_DTYPE]
    scale = np.float32(D ** -0.5)
    wqT = np.ascontiguousarray((q_w * scale).T).astype(mmnp)
    wkT = np.ascontiguousarray(k_w.T).astype(mmnp)
    vwT0 = np.zeros((C, CV), np.float32)
    vT = v_w.T.astype(np.float32)
    for h in range(H):
        vwT0[:, h * D1:h * D1 + D] = vT[:, h * D:(h + 1) * D]
    pwT = np.ascontiguousarray(proj_w.T).astype(mmnp)
    in_maps = []
    for b in range(NCORES):
        xT = np.ascontiguousarray(x[b].T).astype(mmnp)
        # fold head_mask^2 into this core's V weights (ones cols stay 0->1)
        vwT = vwT0.copy()
        for h in range(H):
            vwT[:, h * D1:h * D1 + D] *= head_mask[b, h] ** 2
        in_maps.append(
            {"xT": xT, "wqT": wqT, "wkT": wkT, "vwT": vwT.astype(mmnp),
             "pwT": pwT}
        )
    return in_maps


def _run(inputs, trace=False):
    from concourse.bass_utils import run_bass_kernel_spmd

    x = np.asarray(inputs["x"], np.float32)
    head_mask = np.asarray(inputs["head_mask"], np.float32)
    in_maps = _prep_inputs(
        x,
        head_mask,
        np.asarray(inputs["q_w"], np.float32),
        np.asarray(inputs["k_w"], np.float32),
        np.asarray(inputs["v_w"], np.float32),
        np.asarray(inputs["proj_w"], np.float32),
    )
    # biases are zero by construction of this problem (spec fill=zeros);
    # q_b/k_b/v_b/proj_b are validated and otherwise unused.
    for name in ("q_b", "k_b", "v_b", "proj_b"):
        bias = np.asarray(inputs[name])
        if np.abs(bias).max() > 0:
            raise NotImplementedError(f"nonzero {name} not supported")

    if "nc" not in _cache:
        _cache["nc"] = _build()
    nc = _cache["nc"]
    res = run_bass_kernel_spmd(
        nc, in_maps, core_ids=list(range(NCORES)), trace=trace
    )
    out = np.stack([res.results[b]["out"] for b in range(NCORES)], axis=0)
    return out.astype(np.float32), res


def kernel(**inputs):
    out, _ = _run(inputs, trace=False)
    return out


# revision 10
# speedup vs baseline: 1.0140x; 1.0140x over previous
"""Multi-head attention Trainium2 kernel (B=8, N=1024, C=768, H=12, d=64).

Sharding: data-parallel over batch -- core b computes batch element b.

Per-core dataflow (fp16 matmul operands, fp32 PSUM accumulation; fp16 keeps
the PE on its full-clock datapath):
  - host pre-transposes x -> xT [C, N] and all weights -> [in, out] layout,
    folds the 1/sqrt(d) softmax scale into q_w, extends v_w with a zero
    column per head (slot for the softmax-denominator ones trick).
  - Qt = wqT.T @ xT   [C, N]  (transposed layout, heads on partitions)
  - Kt = wkT.T @ xT   [C, N]
  - V' = xT.T @ vwT'  [N, H*65]  (natural layout; col h*65+64 memset to 1.0)
  - per head pair t, token-chunk ch: both heads' transposed scores land in
    one 2-bank PSUM tile st[128, 2, 512]; ONE Exp activation covers the
    pair. The P@V' accumulation runs one m-tile BEHIND the score stream so
    the PE never waits on the just-issued Exp (software pipeline).
    yt'[d'|sum, n] = V'_h.T @ P accumulated over m-tiles; row 64 = colsum
  - per (t, ch): Yt = yt * head_mask[h]^2 / colsum, normalized immediately
    (reciprocal_approx_fast straight off the PSUM colsum rows; partition
    broadcast on GpSimd) so the tail only waits on the final chunk.
  - out = Yt.T @ pwT  [N, C], staged fp16 (ACT-engine eviction), host casts
    back to fp32.
"""

import numpy as np

B, N, C, H, D = 8, 1024, 768, 12, 64
KO = C // 128          # 6 contraction tiles of 128 channels
MT = N // 128          # 8 token tiles
NCH = N // 512         # 2 free-dim chunks of 512
D1 = D + 1             # V' block width per head (64 V cols + 1 ones col)
CV = H * D1            # 780 extended V channels
NCORES = 8

MM_DTYPE = "f16"

_cache = {}


def _build():
    import concourse.bacc as bacc
    import concourse.mybir as mybir
    import concourse.tile as tile

    F32 = mybir.dt.float32
    MMD = {"bf16": mybir.dt.bfloat16, "f16": mybir.dt.float16,
           "f32r": mybir.dt.float32r, "f32": mybir.dt.float32}[MM_DTYPE]
    AF = mybir.ActivationFunctionType

    nc = bacc.Bacc("TRN2", target_bir_lowering=False, debug=False)

    d_xT = nc.dram_tensor("xT", [C, N], MMD, kind="ExternalInput")
    d_wq = nc.dram_tensor("wqT", [C, C], MMD, kind="ExternalInput")
    d_wk = nc.dram_tensor("wkT", [C, C], MMD, kind="ExternalInput")
    d_wv = nc.dram_tensor("vwT", [C, CV], MMD, kind="ExternalInput")
    d_wp = nc.dram_tensor("pwT", [C, C], MMD, kind="ExternalInput")
    d_out = nc.dram_tensor("out", [N, C], MMD, kind="ExternalOutput")

    r_xT = d_xT.ap().rearrange("(ko p) n -> p ko n", p=128)
    r_wq = d_wq.ap().rearrange("(ko p) m -> p ko m", p=128)
    r_wk = d_wk.ap().rearrange("(ko p) m -> p ko m", p=128)
    r_wv = d_wv.ap().rearrange("(ko p) m -> p ko m", p=128)
    r_wp = d_wp.ap().rearrange("(ko p) m -> p ko m", p=128)
    r_out = d_out.ap().rearrange("(mt p) c -> mt p c", p=128)

    with tile.TileContext(nc) as tc:
        with (
            tc.tile_pool(name="xw", bufs=1) as xw,          # xT, vwT, wp (resident)
            tc.tile_pool(name="wq", bufs=3) as wqp,         # streamed weight blocks
            tc.tile_pool(name="wk", bufs=3) as wkp,
            tc.tile_pool(name="qt", bufs=3) as qtp,         # Qt/Kt streamed per pair
            tc.tile_pool(name="kt", bufs=3) as ktp,
            tc.tile_pool(name="vp", bufs=8) as vpp,         # V' all 8 token tiles
            tc.tile_pool(name="yt", bufs=6) as ytp,         # Yt all 6 channel tiles
            tc.tile_pool(name="pp", bufs=9) as ppp,         # P = exp(St), paired
            tc.tile_pool(name="cs", bufs=4) as csp,         # recip rows
            tc.tile_pool(name="bc", bufs=3) as bcp,         # broadcast tiles
            tc.tile_pool(name="ob", bufs=2) as obp,         # output staging
            tc.tile_pool(name="mm", bufs=2, space="PSUM") as mmp,
            tc.tile_pool(name="st", bufs=2, space="PSUM") as stp,
            tc.tile_pool(name="ya", bufs=2, space="PSUM") as yap,
        ):
            # ---- resident tiles ----
            t_x = xw.tile([128, KO, N], MMD, tag="x")
            t_wv = xw.tile([128, KO, CV], MMD, tag="wv")
            t_wp = xw.tile([128, KO, C], MMD, tag="wpf")

            def make_qk(t, dma_engine):
                """DMA the weight blocks for channel tile t and return
                (t_q, t_k, units) where units are deferred emitters, each
                HALF a PSUM accumulation group (3 matmuls; 2nd half also
                evicts)."""
                t_wqb = wqp.tile([128, KO, 128], MMD, tag="wq", name=f"wqb{t}")
                dma_engine.dma_start(
                    out=t_wqb[:], in_=r_wq[:, :, t * 128:(t + 1) * 128]
                )
                t_wkb = wkp.tile([128, KO, 128], MMD, tag="wk", name=f"wkb{t}")
                dma_engine.dma_start(
                    out=t_wkb[:], in_=r_wk[:, :, t * 128:(t + 1) * 128]
                )
                t_q = qtp.tile([128, N], MMD, tag="qt", name=f"q{t}")
                t_k = ktp.tile([128, N], MMD, tag="kt", name=f"k{t}")

                def unit(wsrc, dst, ch, nm):
                    nsl = slice(ch * 512, (ch + 1) * 512)
                    state = {}

                    def part_a():
                        ps = mmp.tile([128, 512], F32, tag="mm", name=nm)
                        state["ps"] = ps
                        for ko in range(3):
                            nc.tensor.matmul(
                                ps[:], wsrc[:, ko, :], t_x[:, ko, nsl],
                                start=(ko == 0), stop=False,
                            )

                    def part_b():
                        ps = state["ps"]
                        for ko in range(3, KO):
                            nc.tensor.matmul(
                                ps[:], wsrc[:, ko, :], t_x[:, ko, nsl],
                                start=False, stop=(ko == KO - 1),
                            )
                        nc.vector.tensor_copy(dst[:, nsl], ps[:])

                    return [part_a, part_b]

                units = []
                units += unit(t_wqb, t_q, 0, f"pq{t}a")
                units += unit(t_wkb, t_k, 0, f"pk{t}a")
                units += unit(t_wqb, t_q, 1, f"pq{t}b")
                units += unit(t_wkb, t_k, 1, f"pk{t}b")
                return t_q, t_k, units

            # ---- head DMAs, spread across idle engine queues so configs
            # issue in parallel: first-needed operands first ----
            t_q, t_k, units0 = make_qk(0, nc.sync)
            # x: 4 chunks keyed to consumption order (QK proj ch0 ko-halves
            # first)
            nc.scalar.dma_start(out=t_x[:, 0:3, 0:512], in_=r_xT[:, 0:3, 0:512])
            nc.sync.dma_start(out=t_x[:, 3:6, 0:512], in_=r_xT[:, 3:6, 0:512])
            nc.scalar.dma_start(out=t_x[:, 0:3, 512:1024],
                                in_=r_xT[:, 0:3, 512:1024])
            nc.gpsimd.dma_start(out=t_x[:, 3:6, 512:1024],
                                in_=r_xT[:, 3:6, 512:1024])
            nc.gpsimd.dma_start(out=t_wv[:, 0:3, :], in_=r_wv[:, 0:3, :])
            nc.gpsimd.dma_start(out=t_wv[:, 3:6, :], in_=r_wv[:, 3:6, :])
            # wp is not needed until the very end; its DMA is issued after
            # pair 0 so it never competes with the critical head loads

            # pair-0 ch0 projections run first; ch1 projections are woven
            # into the ch0 attention stream (their x chunks land later)
            units0[0]()  # pq0a part_a
            units0[1]()  # pq0a part_b
            units0[2]()  # pk0a part_a
            units0[3]()  # pk0a part_b
            qk_tiles = {0: (t_q, t_k)}
            created = 0
            pend = [(0, u) for u in units0[4:]]  # pq0b + pk0b halves

            # ---- V' projection units (weavable): V'[n, cv] = xT.T @ vwT ----
            t_v = [vpp.tile([128, CV], MMD, tag="v", name=f"v{mt}")
                   for mt in range(MT)]
            vch = [(0, 390), (390, 390)]

            def v_unit(mt):
                tv = t_v[mt]
                state = {}

                def part_a():
                    ps = mmp.tile([128, 512], F32, tag="mm", name=f"v{mt}a")
                    state["ps"] = ps
                    for ko in range(KO):
                        nc.tensor.matmul(
                            ps[:, :390], t_x[:, ko, mt * 128:(mt + 1) * 128],
                            t_wv[:, ko, 0:390],
                            start=(ko == 0), stop=(ko == KO - 1),
                        )
                    nc.vector.tensor_copy(tv[:, 0:390], ps[:, :390])

                def part_b():
                    ps = mmp.tile([128, 512], F32, tag="mm", name=f"v{mt}b")
                    for ko in range(KO):
                        nc.tensor.matmul(
                            ps[:, :390], t_x[:, ko, mt * 128:(mt + 1) * 128],
                            t_wv[:, ko, 390:780],
                            start=(ko == 0), stop=(ko == KO - 1),
                        )
                    nc.vector.tensor_copy(tv[:, 390:780], ps[:, :390])
                    ones_cols = tv[:].rearrange(
                        "p (h e) -> p h e", e=D1)[:, :, D:D + 1]
                    nc.vector.memset(ones_cols, 1.0)

                return [part_a, part_b]

            # all V' units weave into pair 0's attention stream; pair 0's
            # ch0 PV wave is deferred until after its full score wave, by
            # which time the early V' tiles have landed
            for mt in range(MT):
                pend.extend((0, u) for u in v_unit(mt))

            t_yt = [ytp.tile([128, N], MMD, tag="yt", name=f"yt{i}")
                    for i in range(KO)]

            # ---- per channel-tile: attention pair with PV software-pipelined
            # one m-tile behind the score stream; projection half-groups for
            # later pairs woven in as PE filler ----

            for t in range(KO):
                cap = 1 if t == 0 else min(t + 2, KO - 1)
                while created < cap:
                    created += 1
                    q_, k_, us = make_qk(created, nc.gpsimd)
                    qk_tiles[created] = (q_, k_)
                    pend.extend((created, u) for u in us)
                if t == 1:
                    nc.gpsimd.dma_start(out=t_wp[:], in_=r_wp[:])

                for ch in range(NCH):
                    nsl = slice(ch * 512, (ch + 1) * 512)
                    yt0 = yap.tile([D1, 512], F32, tag="ya", name=f"ya{t}{ch}0")
                    yt1 = yap.tile([D1, 512], F32, tag="ya", name=f"ya{t}{ch}1")
                    p_tiles = [None] * MT
                    # pair-0 ch0 runs its whole score wave before any PV
                    # (the V' tiles PV needs are still streaming in); the
                    # steady state runs PV one m-tile behind the scores
                    lag = MT if (t == 0 and ch == 0) else 1
                    for step in range(MT + lag):
                        if step < MT:
                            mt = step
                            msl = slice(mt * 128, (mt + 1) * 128)
                            st = stp.tile([128, 2, 512], F32, tag="st",
                                          name=f"st{t}{ch}{mt}")
                            nc.tensor.matmul(
                                st[:, 0, :], t_k[0:64, msl], t_q[0:64, nsl],
                                start=True, stop=True, tile_position=(0, 0),
                            )
                            nc.tensor.matmul(
                                st[:, 1, :], t_k[64:128, msl], t_q[64:128, nsl],
                                start=True, stop=True, tile_position=(64, 0),
                            )
                            p = ppp.tile([128, 2, 512], MMD, tag="p",
                                         name=f"p{t}{ch}{mt}")
                            nc.scalar.activation(p[:], st[:], AF.Exp)
                            p_tiles[mt] = p
                        if step >= lag:
                            mt = step - lag
                            p = p_tiles[mt]
                            nc.tensor.matmul(
                                yt0[:], t_v[mt][:, (2 * t) * D1:(2 * t + 1) * D1],
                                p[:, 0, :], start=(mt == 0), stop=(mt == MT - 1),
                            )
                            nc.tensor.matmul(
                                yt1[:],
                                t_v[mt][:, (2 * t + 1) * D1:(2 * t + 2) * D1],
                                p[:, 1, :], start=(mt == 0), stop=(mt == MT - 1),
                            )
                        # weave projection half-groups into the stream
                        # (double rate during pair 0 to finish V' in time)
                        npop = 2 if t == 0 else 1
                        for _ in range(npop):
                            if pend:
                                pend.pop(0)[1]()
                    # reciprocals straight off the PSUM colsum rows, then
                    # evict + normalize this chunk on DVE/GpSimd while the
                    # PE streams on
                    t_cs = csp.tile([1, 2, 512], F32, tag="cs", name=f"cs{t}{ch}")
                    nc.vector.tensor_copy(t_cs[0:1, 0, :], yt0[D:D1, :])
                    nc.vector.tensor_copy(t_cs[0:1, 1, :], yt1[D:D1, :])
                    t_rc = csp.tile([1, 2, 512], F32, tag="rc", name=f"rc{t}{ch}")
                    nc.vector.reciprocal_approx_fast(t_rc[:], t_cs[:])
                    nc.vector.tensor_copy(t_yt[t][0:64, nsl], yt0[0:D, :])
                    nc.vector.tensor_copy(t_yt[t][64:128, nsl], yt1[0:D, :])
                    for hp in range(2):
                        psl = slice(hp * 64, hp * 64 + 64)
                        t_bc = bcp.tile([128, 512], F32, tag="bc",
                                        name=f"bc{t}{ch}{hp}")
                        nc.gpsimd.partition_broadcast(
                            t_bc[:], t_rc[0:1, hp, :]
                        )
                        nc.vector.tensor_mul(
                            t_yt[t][psl, nsl], t_yt[t][psl, nsl], t_bc[psl, :]
                        )
                # next pair's projections must be complete before it starts
                for i, u in [pu for pu in pend if pu[0] == t + 1]:
                    u()
                pend = [pu for pu in pend if pu[0] != t + 1]
                if t + 1 < KO:
                    t_q, t_k = qk_tiles[t + 1]

            # ---- output projection: out[n, c] = Yt.T @ pwT; yt stationary
            # shared by both output chunks (halves the LDWEIGHTS); eviction
            # on the ACT engine (Copy shares the Exp table) ----
            for mt in range(MT):
                t_o = obp.tile([128, C], MMD, tag="ob", name=f"ob{mt}")
                msl = slice(mt * 128, (mt + 1) * 128)
                psA = mmp.tile([128, 512], F32, tag="mm", name=f"poA{mt}")
                psB = mmp.tile([128, 512], F32, tag="mm", name=f"poB{mt}")
                for t in range(KO):
                    nc.tensor.matmul(
                        psA[:], t_yt[t][:, msl], t_wp[:, t, 0:512],
                        start=(t == 0), stop=(t == KO - 1),
                    )
                    nc.tensor.matmul(
                        psB[:, 0:256], t_yt[t][:, msl], t_wp[:, t, 512:768],
                        start=(t == 0), stop=(t == KO - 1),
                    )
                nc.vector.tensor_copy(t_o[:, 0:512], psA[:])
                nc.vector.tensor_copy(t_o[:, 512:768], psB[:, 0:256])
                nc.sync.dma_start(out=r_out[mt, :, :], in_=t_o[:])

    nc.compile()
    return nc


def _prep_inputs(x, head_mask, q_w, k_w, v_w, proj_w):
    import ml_dtypes

    mmnp = {"bf16": ml_dtypes.bfloat16, "f16": np.float16,
            "f32r": np.float32, "f32": np.float32}[MM_DTYPE]
    scale = np.float32(D ** -0.5)
    wqT = np.ascontiguousarray((q_w * scale).T).astype(mmnp)
    wkT = np.ascontiguousarray(k_w.T).astype(mmnp)
    vwT0 = np.zeros((C, CV), np.float32)
    vT = v_w.T.astype(np.float32)
    for h in range(H):
        vwT0[:, h * D1:h * D1 + D] = vT[:, h * D:(h + 1) * D]
    pwT = np.ascontiguousarray(proj_w.T).astype(mmnp)
    in_maps = []
    for b in range(NCORES):
        xT = np.ascontiguousarray(x[b].T).astype(mmnp)
        # fold head_mask^2 into this core's V weights (ones cols stay 0->1)
        vwT = vwT0.copy()
        for h in range(H):
            vwT[:, h * D1:h * D1 + D] *= head_mask[b, h] ** 2
        in_maps.append(
            {"xT": xT, "wqT": wqT, "wkT": wkT, "vwT": vwT.astype(mmnp),
             "pwT": pwT}
        )
    return in_maps


def _run(inputs, trace=False):
    from concourse.bass_utils import run_bass_kernel_spmd

    x = np.asarray(inputs["x"], np.float32)
    head_mask = np.asarray(inputs["head_mask"], np.float32)
    in_maps = _prep_inputs(
        x,
        head_mask,
        np.asarray(inputs["q_w"], np.float32),
        np.asarray(inputs["k_w"], np.float32),
        np.asarray(inputs["v_w"], np.float32),
        np.asarray(inputs["proj_w"], np.float32),
    )
    # biases are zero by construction of this problem (spec fill=zeros);
    # q_b/k_b/v_b/proj_b are validated and otherwise unused.
    for name in ("q_b", "k_b", "v_b", "proj_b"):
        bias = np.asarray(inputs[name])
        if np.abs(bias).max() > 0:
            raise NotImplementedError(f"nonzero {name} not supported")

    if "nc" not in _cache:
        _cache["nc"] = _build()
    nc = _cache["nc"]
    res = run_bass_kernel_spmd(
        nc, in_maps, core_ids=list(range(NCORES)), trace=trace
    )
    out = np.stack([res.results[b]["out"] for b in range(NCORES)], axis=0)
    return out.astype(np.float32), res


def kernel(**inputs):
    out, _ = _run(inputs, trace=False)
    return out


# revision 19
# speedup vs baseline: 1.0369x; 1.0226x over previous
"""Multi-head attention Trainium2 kernel (B=8, N=1024, C=768, H=12, d=64).

Sharding: data-parallel over batch -- core b computes batch element b.

Per-core dataflow (fp16 matmul operands, fp32 PSUM accumulation; fp16 keeps
the PE on its full-clock datapath):
  - host pre-transposes x -> xT [C, N] and all weights -> [in, out] layout,
    folds the 1/sqrt(d) softmax scale into q_w, extends v_w with a zero
    column per head (slot for the softmax-denominator ones trick).
  - Qt = wqT.T @ xT   [C, N]  (transposed layout, heads on partitions)
  - Kt = wkT.T @ xT   [C, N]
  - V' = xT.T @ vwT'  [N, H*65]  (natural layout; col h*65+64 memset to 1.0)
  - per head pair t, token-chunk ch: both heads' transposed scores land in
    one 2-bank PSUM tile st[128, 2, 512]; ONE Exp activation covers the
    pair. The P@V' accumulation runs one m-tile BEHIND the score stream so
    the PE never waits on the just-issued Exp (software pipeline).
    yt'[d'|sum, n] = V'_h.T @ P accumulated over m-tiles; row 64 = colsum
  - per (t, ch): Yt = yt * head_mask[h]^2 / colsum, normalized immediately
    (reciprocal_approx_fast straight off the PSUM colsum rows; partition
    broadcast on GpSimd) so the tail only waits on the final chunk.
  - out = Yt.T @ pwT  [N, C], staged fp16 (ACT-engine eviction), host casts
    back to fp32.
"""

import numpy as np

B, N, C, H, D = 8, 1024, 768, 12, 64
KO = C // 128          # 6 contraction tiles of 128 channels
MT = N // 128          # 8 token tiles
NCH = N // 512         # 2 free-dim chunks of 512
D1 = D + 1             # V' block width per head (64 V cols + 1 ones col)
CV = H * D1            # 780 extended V channels
NCORES = 8

MM_DTYPE = "f16"

_cache = {}


def _build():
    import concourse.bacc as bacc
    import concourse.mybir as mybir
    import concourse.tile as tile

    F32 = mybir.dt.float32
    MMD = {"bf16": mybir.dt.bfloat16, "f16": mybir.dt.float16,
           "f32r": mybir.dt.float32r, "f32": mybir.dt.float32}[MM_DTYPE]
    AF = mybir.ActivationFunctionType

    nc = bacc.Bacc("TRN2", target_bir_lowering=False, debug=False)

    d_xT = nc.dram_tensor("xT", [C, N], MMD, kind="ExternalInput")
    d_wq = nc.dram_tensor("wqT", [C, C], MMD, kind="ExternalInput")
    d_wk = nc.dram_tensor("wkT", [C, C], MMD, kind="ExternalInput")
    d_wv = nc.dram_tensor("vwT", [C, CV], MMD, kind="ExternalInput")
    d_wp = nc.dram_tensor("pwT", [C, C], MMD, kind="ExternalInput")
    d_out = nc.dram_tensor("out", [N, C], MMD, kind="ExternalOutput")

    r_xT = d_xT.ap().rearrange("(ko p) n -> p ko n", p=128)
    r_wq = d_wq.ap().rearrange("(ko p) m -> p ko m", p=128)
    r_wk = d_wk.ap().rearrange("(ko p) m -> p ko m", p=128)
    r_wv = d_wv.ap().rearrange("(ko p) m -> p ko m", p=128)
    r_wp = d_wp.ap().rearrange("(ko p) m -> p ko m", p=128)
    r_out = d_out.ap().rearrange("(mt p) c -> mt p c", p=128)

    with tile.TileContext(nc) as tc:
        with (
            tc.tile_pool(name="xw", bufs=1) as xw,          # xT, vwT, wp (resident)
            tc.tile_pool(name="wq", bufs=3) as wqp,         # streamed weight blocks
            tc.tile_pool(name="wk", bufs=3) as wkp,
            tc.tile_pool(name="qt", bufs=3) as qtp,         # Qt/Kt streamed per pair
            tc.tile_pool(name="kt", bufs=3) as ktp,
            tc.tile_pool(name="vp", bufs=8) as vpp,         # V' all 8 token tiles
            tc.tile_pool(name="yt", bufs=6) as ytp,         # Yt all 6 channel tiles
            tc.tile_pool(name="pp", bufs=9) as ppp,         # P = exp(St), paired
            tc.tile_pool(name="cs", bufs=4) as csp,         # recip rows
            tc.tile_pool(name="bc", bufs=3) as bcp,         # broadcast tiles
            tc.tile_pool(name="ob", bufs=2) as obp,         # output staging
            tc.tile_pool(name="mm", bufs=2, space="PSUM") as mmp,
            tc.tile_pool(name="st", bufs=2, space="PSUM") as stp,
            tc.tile_pool(name="ya", bufs=2, space="PSUM") as yap,
        ):
            # ---- resident tiles ----
            t_x = xw.tile([128, KO, N], MMD, tag="x")
            t_wv = xw.tile([128, KO, CV], MMD, tag="wv")
            t_wp = xw.tile([128, KO, C], MMD, tag="wpf")

            def make_qk(t, dma_engine, dma_engine2=None):
                """DMA the weight blocks for channel tile t and return
                (t_q, t_k, units) where units are deferred emitters, each
                HALF a PSUM accumulation group (3 matmuls; 2nd half also
                evicts)."""
                t_wqb = wqp.tile([128, KO, 128], MMD, tag="wq", name=f"wqb{t}")
                dma_engine.dma_start(
                    out=t_wqb[:], in_=r_wq[:, :, t * 128:(t + 1) * 128]
                )
                t_wkb = wkp.tile([128, KO, 128], MMD, tag="wk", name=f"wkb{t}")
                (dma_engine2 or dma_engine).dma_start(
                    out=t_wkb[:], in_=r_wk[:, :, t * 128:(t + 1) * 128]
                )
                t_q = qtp.tile([128, N], MMD, tag="qt", name=f"q{t}")
                t_k = ktp.tile([128, N], MMD, tag="kt", name=f"k{t}")

                def unit(wsrc, dst, ch, nm):
                    nsl = slice(ch * 512, (ch + 1) * 512)
                    state = {}

                    def part_a():
                        ps = mmp.tile([128, 512], F32, tag="mm", name=nm)
                        state["ps"] = ps
                        for ko in range(3):
                            nc.tensor.matmul(
                                ps[:], wsrc[:, ko, :], t_x[:, ko, nsl],
                                start=(ko == 0), stop=False,
                            )

                    def part_b():
                        ps = state["ps"]
                        for ko in range(3, KO):
                            nc.tensor.matmul(
                                ps[:], wsrc[:, ko, :], t_x[:, ko, nsl],
                                start=False, stop=(ko == KO - 1),
                            )
                        nc.vector.tensor_copy(dst[:, nsl], ps[:])

                    return [part_a, part_b]

                units = []
                units += unit(t_wqb, t_q, 0, f"pq{t}a")
                units += unit(t_wkb, t_k, 0, f"pk{t}a")
                units += unit(t_wqb, t_q, 1, f"pq{t}b")
                units += unit(t_wkb, t_k, 1, f"pk{t}b")
                return t_q, t_k, units

            # ---- head DMAs, spread across idle engine queues so configs
            # issue in parallel: first-needed operands first ----
            t_q, t_k, units0 = make_qk(0, nc.sync, nc.scalar)
            # x split finely across queues so the first-needed chunk (QK
            # proj ch0) isn't starved by fair-share bandwidth; wq0/wk0 lead
            # their queues
            nc.sync.dma_start(out=t_x[:, 0:3, 0:256], in_=r_xT[:, 0:3, 0:256])
            nc.scalar.dma_start(out=t_x[:, 0:3, 256:512],
                                in_=r_xT[:, 0:3, 256:512])
            nc.sync.dma_start(out=t_x[:, 3:6, 0:256], in_=r_xT[:, 3:6, 0:256])
            nc.scalar.dma_start(out=t_x[:, 3:6, 256:512],
                                in_=r_xT[:, 3:6, 256:512])
            nc.gpsimd.dma_start(out=t_x[:, :, 512:1024],
                                in_=r_xT[:, :, 512:1024])
            nc.gpsimd.dma_start(out=t_wv[:, 0:3, :], in_=r_wv[:, 0:3, :])
            nc.gpsimd.dma_start(out=t_wv[:, 3:6, :], in_=r_wv[:, 3:6, :])
            # wp is not needed until the very end; its DMA is issued after
            # pair 0 so it never competes with the critical head loads

            # pair-0 ch0 projections run first; ch1 projections are woven
            # into the ch0 attention stream (their x chunks land later)
            units0[0]()  # pq0a part_a
            units0[1]()  # pq0a part_b
            units0[2]()  # pk0a part_a
            units0[3]()  # pk0a part_b
            qk_tiles = {0: (t_q, t_k)}
            created = 0
            pend = []  # (need_by_tile_idx, deferred emitter)

            # ---- V' projection units (weavable): V'[n, cv] = xT.T @ vwT ----
            t_v = [vpp.tile([128, CV], MMD, tag="v", name=f"v{mt}")
                   for mt in range(MT)]
            vch = [(0, 390), (390, 390)]

            def v_unit(mt):
                tv = t_v[mt]
                state = {}

                def part_a():
                    ps = mmp.tile([128, 512], F32, tag="mm", name=f"v{mt}a")
                    state["ps"] = ps
                    for ko in range(KO):
                        nc.tensor.matmul(
                            ps[:, :390], t_x[:, ko, mt * 128:(mt + 1) * 128],
                            t_wv[:, ko, 0:390],
                            start=(ko == 0), stop=(ko == KO - 1),
                        )
                    nc.vector.tensor_copy(tv[:, 0:390], ps[:, :390])

                def part_b():
                    ps = mmp.tile([128, 512], F32, tag="mm", name=f"v{mt}b")
                    for ko in range(KO):
                        nc.tensor.matmul(
                            ps[:, :390], t_x[:, ko, mt * 128:(mt + 1) * 128],
                            t_wv[:, ko, 390:780],
                            start=(ko == 0), stop=(ko == KO - 1),
                        )
                    nc.vector.tensor_copy(tv[:, 390:780], ps[:, :390])
                    ones_cols = tv[:].rearrange(
                        "p (h e) -> p h e", e=D1)[:, :, D:D + 1]
                    nc.vector.memset(ones_cols, 1.0)

                return [part_a, part_b]

            # all V' units weave into pair 0's attention stream; pair 0's
            # ch0 PV wave is deferred until after its full score wave, by
            # which time the early V' tiles have landed. ch1's Q/K
            # projections follow the V' units (x ch1 lands late).
            # pk0b first (K ch1 is read by every score m-tile >= 4), then
            # pq0b (only ch1 queries need it), then the V' units
            pend.extend((0, u) for u in (units0[6], units0[7],
                                         units0[4], units0[5]))
            for mt in range(MT):
                pend.extend((0, u) for u in v_unit(mt))

            t_yt = [ytp.tile([128, N], MMD, tag="yt", name=f"yt{i}")
                    for i in range(KO)]

            # ---- per channel-tile: attention pair with PV software-pipelined
            # one m-tile behind the score stream; projection half-groups for
            # later pairs woven in as PE filler ----

            for t in range(KO):
                cap = 1 if t == 0 else min(t + 2, KO - 1)
                while created < cap:
                    created += 1
                    q_, k_, us = make_qk(created, nc.gpsimd)
                    qk_tiles[created] = (q_, k_)
                    pend.extend((created, u) for u in us)
                if t == 1:
                    nc.gpsimd.dma_start(out=t_wp[:], in_=r_wp[:])

                # the last pair runs ch1 first so ch0 (whose tokens the
                # output projection consumes first) is normalized last but
                # the projection starts on ch1 tokens immediately
                ch_order = [1, 0] if t == KO - 1 else [0, 1]
                for ch in ch_order:
                    nsl = slice(ch * 512, (ch + 1) * 512)
                    yt0 = yap.tile([D1, 512], F32, tag="ya", name=f"ya{t}{ch}0")
                    yt1 = yap.tile([D1, 512], F32, tag="ya", name=f"ya{t}{ch}1")
                    p_tiles = [None] * MT
                    # pair-0 ch0 runs its whole score wave before any PV
                    # (the V' tiles PV needs are still streaming in); the
                    # steady state runs PV one m-tile behind the scores
                    first = (t == 0 and ch == ch_order[0])
                    lag = MT if first else 1
                    for step in range(MT + lag):
                        if step < MT:
                            mt = step
                            msl = slice(mt * 128, (mt + 1) * 128)
                            st = stp.tile([128, 2, 512], F32, tag="st",
                                          name=f"st{t}{ch}{mt}")
                            nc.tensor.matmul(
                                st[:, 0, :], t_k[0:64, msl], t_q[0:64, nsl],
                                start=True, stop=True, tile_position=(0, 0),
                            )
                            nc.tensor.matmul(
                                st[:, 1, :], t_k[64:128, msl], t_q[64:128, nsl],
                                start=True, stop=True, tile_position=(64, 0),
                            )
                            p = ppp.tile([128, 2, 512], MMD, tag="p",
                                         name=f"p{t}{ch}{mt}")
                            nc.scalar.activation(p[:], st[:], AF.Exp)
                            p_tiles[mt] = p
                            # pk0b lands here: K ch1 must be complete
                            # before the step-4 score reads it
                            if first and step in (2, 3) and pend:
                                pend.pop(0)[1]()
                        if step >= lag:
                            mt = step - lag
                            # during pair-0's PV wave the remaining setup
                            # units pop just-in-time BEFORE the PV that
                            # consumes them
                            if first:
                                for _ in range(4):
                                    if pend:
                                        pend.pop(0)[1]()
                            p = p_tiles[mt]
                            nc.tensor.matmul(
                                yt0[:], t_v[mt][:, (2 * t) * D1:(2 * t + 1) * D1],
                                p[:, 0, :], start=(mt == 0), stop=(mt == MT - 1),
                            )
                            nc.tensor.matmul(
                                yt1[:],
                                t_v[mt][:, (2 * t + 1) * D1:(2 * t + 2) * D1],
                                p[:, 1, :], start=(mt == 0), stop=(mt == MT - 1),
                            )
                        # weave projection half-groups into the stream
                        if not first and pend:
                            pend.pop(0)[1]()
                    # reciprocals off SBUF copies of the colsum rows, then
                    # evict + normalize this chunk on DVE/GpSimd while the
                    # PE streams on
                    t_cs = csp.tile([1, 2, 512], F32, tag="cs", name=f"cs{t}{ch}")
                    nc.vector.tensor_copy(t_cs[0:1, 0, :], yt0[D:D1, :])
                    nc.vector.tensor_copy(t_cs[0:1, 1, :], yt1[D:D1, :])
                    t_rc = csp.tile([1, 2, 512], F32, tag="rc", name=f"rc{t}{ch}")
                    nc.vector.reciprocal_approx_fast(t_rc[0:1, 0, :],
                                                     t_cs[0:1, 0, :])
                    nc.vector.reciprocal_approx_fast(t_rc[0:1, 1, :],
                                                     t_cs[0:1, 1, :])
                    nc.vector.tensor_copy(t_yt[t][0:64, nsl], yt0[0:D, :])
                    nc.vector.tensor_copy(t_yt[t][64:128, nsl], yt1[0:D, :])
                    for hp in range(2):
                        psl = slice(hp * 64, hp * 64 + 64)
                        t_bc = bcp.tile([128, 512], F32, tag="bc",
                                        name=f"bc{t}{ch}{hp}")
                        nc.gpsimd.partition_broadcast(
                            t_bc[:], t_rc[0:1, hp, :]
                        )
                        nc.vector.tensor_mul(
                            t_yt[t][psl, nsl], t_yt[t][psl, nsl], t_bc[psl, :]
                        )
                    # anything still pending that this pair needs must land
                    # before the next chunk reads it
                    due = [pu for pu in pend if pu[0] <= t]
                    if due:
                        for i, u in due:
                            u()
                        pend = [pu for pu in pend if pu[0] > t]
                # next pair's projections must be complete before it starts
                for i, u in [pu for pu in pend if pu[0] == t + 1]:
                    u()
                pend = [pu for pu in pend if pu[0] != t + 1]
                if t + 1 < KO:
                    t_q, t_k = qk_tiles[t + 1]

            # ---- output projection: out[n, c] = Yt.T @ pwT; yt stationary
            # shared by both output chunks (halves the LDWEIGHTS). ch1
            # tokens (mt 4-7) first: the last pair normalizes them first.
            # Accumulators split over the st + mm pools so consecutive
            # token tiles overlap with the evictions. ----
            for mt in [4, 5, 6, 7, 0, 1, 2, 3]:
                t_o = obp.tile([128, C], MMD, tag="ob", name=f"ob{mt}")
                msl = slice(mt * 128, (mt + 1) * 128)
                psA = stp.tile([128, 512], F32, tag="st", name=f"poA{mt}")
                psB = mmp.tile([128, 512], F32, tag="mm", name=f"poB{mt}")
                for t in range(KO):
                    nc.tensor.matmul(
                        psA[:], t_yt[t][:, msl], t_wp[:, t, 0:512],
                        start=(t == 0), stop=(t == KO - 1),
                    )
                    nc.tensor.matmul(
                        psB[:, 0:256], t_yt[t][:, msl], t_wp[:, t, 512:768],
                        start=(t == 0), stop=(t == KO - 1),
                    )
                nc.vector.tensor_copy(t_o[:, 0:512], psA[:])
                nc.vector.tensor_copy(t_o[:, 512:768], psB[:, 0:256])
                nc.sync.dma_start(out=r_out[mt, :, :], in_=t_o[:])

    nc.compile()
    return nc


def _prep_inputs(x, head_mask, q_w, k_w, v_w, proj_w):
    import ml_dtypes

    mmnp = {"bf16": ml_dtypes.bfloat16, "f16": np.float16,
            "f32r": np.float32, "f32": np.float32}[MM_DTYPE]
    scale = np.float32(D ** -0.5)
    wqT = np.ascontiguousarray((q_w * scale).T).astype(mmnp)
    wkT = np.ascontiguousarray(k_w.T).astype(mmnp)
    vwT0 = np.zeros((C, CV), np.float32)
    vT = v_w.T.astype(np.float32)
    for h in range(H):
        vwT0[:, h * D1:h * D1 + D] = vT[:, h * D:(h + 1) * D]
    pwT = np.ascontiguousarray(proj_w.T).astype(mmnp)
    in_maps = []
    for b in range(NCORES):
        xT = np.ascontiguousarray(x[b].T).astype(mmnp)
        # fold head_mask^2 into this core's V weights (ones cols stay 0->1)
        vwT = vwT0.copy()
        for h in range(H):
            vwT[:, h * D1:h * D1 + D] *= head_mask[b, h] ** 2
        in_maps.append(
            {"xT": xT, "wqT": wqT, "wkT": wkT, "vwT": vwT.astype(mmnp),
             "pwT": pwT}
        )
    return in_maps


def _run(inputs, trace=False):
    from concourse.bass_utils import run_bass_kernel_spmd

    x = np.asarray(inputs["x"], np.float32)
    head_mask = np.asarray(inputs["head_mask"], np.float32)
    in_maps = _prep_inputs(
        x,
        head_mask,
        np.asarray(inputs["q_w"], np.float32),
        np.asarray(inputs["k_w"], np.float32),
        np.asarray(inputs["v_w"], np.float32),
        np.asarray(inputs["proj_w"], np.float32),
    )
    # biases are zero by construction of this problem (spec fill=zeros);
    # q_b/k_b/v_b/proj_b are validated and otherwise unused.
    for name in ("q_b", "k_b", "v_b", "proj_b"):
        bias = np.asarray(inputs[name])
        if np.abs(bias).max() > 0:
            raise NotImplementedError(f"nonzero {name} not supported")

    if "nc" not in _cache:
        _cache["nc"] = _build()
    nc = _cache["nc"]
    res = run_bass_kernel_spmd(
        nc, in_maps, core_ids=list(range(NCORES)), trace=trace
    )
    out = np.stack([res.results[b]["out"] for b in range(NCORES)], axis=0)
    return out.astype(np.float32), res


def kernel(**inputs):
    out, _ = _run(inputs, trace=False)
    return out


# revision 24
# speedup vs baseline: 1.0528x; 1.0153x over previous
"""Multi-head attention Trainium2 kernel (B=8, N=1024, C=768, H=12, d=64).

Sharding: data-parallel over batch -- core b computes batch element b.

Per-core dataflow (fp16 matmul operands, fp32 PSUM accumulation; fp16 keeps
the PE on its full-clock datapath):
  - host pre-transposes x -> xT [C, N] and all weights -> [in, out] layout,
    folds the 1/sqrt(d) softmax scale into q_w, extends v_w with a zero
    column per head (slot for the softmax-denominator ones trick).
  - Qt = wqT.T @ xT   [C, N]  (transposed layout, heads on partitions)
  - Kt = wkT.T @ xT   [C, N]
  - V' = xT.T @ vwT'  [N, H*65]  (natural layout; col h*65+64 memset to 1.0)
  - per head pair t, token-chunk ch: both heads' transposed scores land in
    one 2-bank PSUM tile st[128, 2, 512]; ONE Exp activation covers the
    pair. The P@V' accumulation runs one m-tile BEHIND the score stream so
    the PE never waits on the just-issued Exp (software pipeline).
    yt'[d'|sum, n] = V'_h.T @ P accumulated over m-tiles; row 64 = colsum
  - per (t, ch): Yt = yt * head_mask[h]^2 / colsum, normalized immediately
    (reciprocal_approx_fast straight off the PSUM colsum rows; partition
    broadcast on GpSimd) so the tail only waits on the final chunk.
  - out = Yt.T @ pwT  [N, C], staged fp16 (ACT-engine eviction), host casts
    back to fp32.
"""

import numpy as np

B, N, C, H, D = 8, 1024, 768, 12, 64
KO = C // 128          # 6 contraction tiles of 128 channels
MT = N // 128          # 8 token tiles
NCH = N // 512         # 2 free-dim chunks of 512
D1 = D + 1             # V' block width per head (64 V cols + 1 ones col)
CV = H * D1            # 780 extended V channels
NCORES = 8

MM_DTYPE = "f16"

_cache = {}


def _build():
    import concourse.bacc as bacc
    import concourse.mybir as mybir
    import concourse.tile as tile

    F32 = mybir.dt.float32
    MMD = {"bf16": mybir.dt.bfloat16, "f16": mybir.dt.float16,
           "f32r": mybir.dt.float32r, "f32": mybir.dt.float32}[MM_DTYPE]
    AF = mybir.ActivationFunctionType

    nc = bacc.Bacc("TRN2", target_bir_lowering=False, debug=False)

    d_xT = nc.dram_tensor("xT", [C, N], MMD, kind="ExternalInput")
    d_wq = nc.dram_tensor("wqT", [C, C], MMD, kind="ExternalInput")
    d_wk = nc.dram_tensor("wkT", [C, C], MMD, kind="ExternalInput")
    d_wv = nc.dram_tensor("vwT", [C, CV], MMD, kind="ExternalInput")
    d_wp = nc.dram_tensor("pwT", [C, C], MMD, kind="ExternalInput")
    d_out = nc.dram_tensor("out", [N, C], MMD, kind="ExternalOutput")

    r_xT = d_xT.ap().rearrange("(ko p) n -> p ko n", p=128)
    r_wq = d_wq.ap().rearrange("(ko p) m -> p ko m", p=128)
    r_wk = d_wk.ap().rearrange("(ko p) m -> p ko m", p=128)
    r_wv = d_wv.ap().rearrange("(ko p) m -> p ko m", p=128)
    r_wp = d_wp.ap().rearrange("(ko p) m -> p ko m", p=128)
    r_out = d_out.ap().rearrange("(mt p) c -> mt p c", p=128)

    with tile.TileContext(nc) as tc:
        with (
            tc.tile_pool(name="xw", bufs=1) as xw,          # xT, vwT, wp (resident)
            tc.tile_pool(name="wq", bufs=3) as wqp,         # streamed weight blocks
            tc.tile_pool(name="wk", bufs=3) as wkp,
            tc.tile_pool(name="qt", bufs=3) as qtp,         # Qt/Kt streamed per pair
            tc.tile_pool(name="kt", bufs=3) as ktp,
            tc.tile_pool(name="vp", bufs=8) as vpp,         # V' all 8 token tiles
            tc.tile_pool(name="yt", bufs=6) as ytp,         # Yt all 6 channel tiles
            tc.tile_pool(name="pp", bufs=9) as ppp,         # P = exp(St), paired
            tc.tile_pool(name="cs", bufs=4) as csp,         # recip rows
            tc.tile_pool(name="bc", bufs=3) as bcp,         # broadcast tiles
            tc.tile_pool(name="ob", bufs=2) as obp,         # output staging
            tc.tile_pool(name="mm", bufs=2, space="PSUM") as mmp,
            tc.tile_pool(name="st", bufs=2, space="PSUM") as stp,
            tc.tile_pool(name="ya", bufs=2, space="PSUM") as yap,
        ):
            # ---- resident tiles. x is FOUR tiles (ko-half x token-half),
            # one DMA each: tile-granular dependency tracking means a
            # shared tile would stall the first projection until ALL x
            # chunks landed. ----
            xt = {(kh, th): xw.tile([128, 3, 512], MMD, tag=f"x{kh}{th}",
                                    name=f"x{kh}{th}")
                  for kh in range(2) for th in range(2)}
            t_wv = xw.tile([128, KO, CV], MMD, tag="wv")
            t_wp = xw.tile([128, KO, C], MMD, tag="wpf")

            def x_ap(ko, col0, w):
                kh, th = ko // 3, col0 // 512
                c = col0 - 512 * th
                return xt[(kh, th)][:, ko - 3 * kh, c:c + w]

            def make_qk(t, dma_engine, dma_engine2=None):
                """DMA the weight blocks for channel tile t and return
                (t_q, t_k, units) where units are deferred emitters, each
                HALF a PSUM accumulation group (3 matmuls; 2nd half also
                evicts)."""
                t_wqb = wqp.tile([128, KO, 128], MMD, tag="wq", name=f"wqb{t}")
                dma_engine.dma_start(
                    out=t_wqb[:], in_=r_wq[:, :, t * 128:(t + 1) * 128]
                )
                t_wkb = wkp.tile([128, KO, 128], MMD, tag="wk", name=f"wkb{t}")
                (dma_engine2 or dma_engine).dma_start(
                    out=t_wkb[:], in_=r_wk[:, :, t * 128:(t + 1) * 128]
                )
                t_q = qtp.tile([128, N], MMD, tag="qt", name=f"q{t}")
                t_k = ktp.tile([128, N], MMD, tag="kt", name=f"k{t}")

                def unit(wsrc, dst, ch, nm):
                    nsl = slice(ch * 512, (ch + 1) * 512)
                    state = {}

                    def part_a():
                        ps = mmp.tile([128, 512], F32, tag="mm", name=nm)
                        state["ps"] = ps
                        for ko in range(3):
                            nc.tensor.matmul(
                                ps[:], wsrc[:, ko, :], x_ap(ko, ch * 512, 512),
                                start=(ko == 0), stop=False,
                            )

                    def part_b():
                        ps = state["ps"]
                        for ko in range(3, KO):
                            nc.tensor.matmul(
                                ps[:], wsrc[:, ko, :], x_ap(ko, ch * 512, 512),
                                start=False, stop=(ko == KO - 1),
                            )
                        nc.vector.tensor_copy(dst[:, nsl], ps[:])

                    return [part_a, part_b]

                units = []
                units += unit(t_wqb, t_q, 0, f"pq{t}a")
                units += unit(t_wkb, t_k, 0, f"pk{t}a")
                units += unit(t_wqb, t_q, 1, f"pq{t}b")
                units += unit(t_wkb, t_k, 1, f"pk{t}b")
                return t_q, t_k, units

            # ---- head DMAs, spread across idle engine queues so configs
            # issue in parallel: first-needed operands first ----
            t_q, t_k, units0 = make_qk(0, nc.sync, nc.scalar)
            # one DMA per x tile; wq0/wk0 lead their queues so the first
            # projection's operands arrive first
            nc.sync.dma_start(out=xt[(0, 0)][:], in_=r_xT[:, 0:3, 0:512])
            nc.scalar.dma_start(out=xt[(1, 0)][:], in_=r_xT[:, 3:6, 0:512])
            nc.gpsimd.dma_start(out=xt[(0, 1)][:], in_=r_xT[:, 0:3, 512:1024])
            nc.gpsimd.dma_start(out=xt[(1, 1)][:], in_=r_xT[:, 3:6, 512:1024])
            nc.gpsimd.dma_start(out=t_wv[:, 0:3, :], in_=r_wv[:, 0:3, :])
            nc.gpsimd.dma_start(out=t_wv[:, 3:6, :], in_=r_wv[:, 3:6, :])
            # wp is not needed until the very end; its DMA is issued after
            # pair 0 so it never competes with the critical head loads

            # pair-0 ch0 projections run first; ch1 projections are woven
            # into the ch0 attention stream (their x chunks land later)
            units0[0]()  # pq0a part_a
            units0[1]()  # pq0a part_b
            units0[2]()  # pk0a part_a
            units0[3]()  # pk0a part_b
            qk_tiles = {0: (t_q, t_k)}
            created = 0
            pend = []  # (need_by_tile_idx, deferred emitter)

            # ---- V' projection units (weavable): V'[n, cv] = xT.T @ vwT ----
            t_v = [vpp.tile([128, CV], MMD, tag="v", name=f"v{mt}")
                   for mt in range(MT)]
            vch = [(0, 390), (390, 390)]

            def v_unit(mt):
                tv = t_v[mt]
                state = {}

                def part_a():
                    ps = mmp.tile([128, 512], F32, tag="mm", name=f"v{mt}a")
                    state["ps"] = ps
                    for ko in range(KO):
                        nc.tensor.matmul(
                            ps[:, :390], x_ap(ko, mt * 128, 128),
                            t_wv[:, ko, 0:390],
                            start=(ko == 0), stop=(ko == KO - 1),
                        )
                    nc.vector.tensor_copy(tv[:, 0:390], ps[:, :390])

                def part_b():
                    ps = mmp.tile([128, 512], F32, tag="mm", name=f"v{mt}b")
                    for ko in range(KO):
                        nc.tensor.matmul(
                            ps[:, :390], x_ap(ko, mt * 128, 128),
                            t_wv[:, ko, 390:780],
                            start=(ko == 0), stop=(ko == KO - 1),
                        )
                    nc.vector.tensor_copy(tv[:, 390:780], ps[:, :390])
                    ones_cols = tv[:].rearrange(
                        "p (h e) -> p h e", e=D1)[:, :, D:D + 1]
                    nc.vector.memset(ones_cols, 1.0)

                return [part_a, part_b]

            # all V' units weave into pair 0's attention stream; pair 0's
            # ch0 PV wave is deferred until after its full score wave, by
            # which time the early V' tiles have landed. ch1's Q/K
            # projections follow the V' units (x ch1 lands late).
            # pk0b first (K ch1 is read by every score m-tile >= 4), then
            # pq0b (only ch1 queries need it), then the V' units
            pend.extend((0, u) for u in (units0[6], units0[7],
                                         units0[4], units0[5]))
            for mt in range(MT):
                pend.extend((0, u) for u in v_unit(mt))

            t_yt = [ytp.tile([128, N], MMD, tag="yt", name=f"yt{i}")
                    for i in range(KO)]

            # ---- per channel-tile: attention pair with PV software-pipelined
            # one m-tile behind the score stream; projection half-groups for
            # later pairs woven in as PE filler ----

            for t in range(KO):
                cap = 1 if t == 0 else min(t + 2, KO - 1)
                while created < cap:
                    created += 1
                    q_, k_, us = make_qk(created, nc.gpsimd)
                    qk_tiles[created] = (q_, k_)
                    pend.extend((created, u) for u in us)
                if t == 1:
                    nc.gpsimd.dma_start(out=t_wp[:], in_=r_wp[:])

                # the last pair runs ch1 first so ch0 (whose tokens the
                # output projection consumes first) is normalized last but
                # the projection starts on ch1 tokens immediately
                ch_order = [1, 0] if t == KO - 1 else [0, 1]
                for ch in ch_order:
                    nsl = slice(ch * 512, (ch + 1) * 512)
                    yt0 = yap.tile([D1, 512], F32, tag="ya", name=f"ya{t}{ch}0")
                    yt1 = yap.tile([D1, 512], F32, tag="ya", name=f"ya{t}{ch}1")
                    p_tiles = [None] * MT
                    # pair-0 ch0 runs its whole score wave before any PV
                    # (the V' tiles PV needs are still streaming in); the
                    # steady state runs PV one m-tile behind the scores
                    first = (t == 0 and ch == ch_order[0])
                    lag = MT if first else 1
                    for step in range(MT + lag):
                        if step < MT:
                            mt = step
                            msl = slice(mt * 128, (mt + 1) * 128)
                            st = stp.tile([128, 2, 512], F32, tag="st",
                                          name=f"st{t}{ch}{mt}")
                            nc.tensor.matmul(
                                st[:, 0, :], t_k[0:64, msl], t_q[0:64, nsl],
                                start=True, stop=True, tile_position=(0, 0),
                            )
                            nc.tensor.matmul(
                                st[:, 1, :], t_k[64:128, msl], t_q[64:128, nsl],
                                start=True, stop=True, tile_position=(64, 0),
                            )
                            p = ppp.tile([128, 2, 512], MMD, tag="p",
                                         name=f"p{t}{ch}{mt}")
                            nc.scalar.activation(p[:], st[:], AF.Exp)
                            p_tiles[mt] = p
                            # pk0b lands here: K ch1 must be complete
                            # before the step-4 score reads it
                            if first and step in (2, 3) and pend:
                                pend.pop(0)[1]()
                        if step >= lag:
                            mt = step - lag
                            # during pair-0's PV wave the remaining setup
                            # units pop just-in-time BEFORE the PV that
                            # consumes them
                            if first:
                                for _ in range(4):
                                    if pend:
                                        pend.pop(0)[1]()
                            p = p_tiles[mt]
                            nc.tensor.matmul(
                                yt0[:], t_v[mt][:, (2 * t) * D1:(2 * t + 1) * D1],
                                p[:, 0, :], start=(mt == 0), stop=(mt == MT - 1),
                            )
                            nc.tensor.matmul(
                                yt1[:],
                                t_v[mt][:, (2 * t + 1) * D1:(2 * t + 2) * D1],
                                p[:, 1, :], start=(mt == 0), stop=(mt == MT - 1),
                            )
                        # weave projection half-groups into the stream
                        if not first and pend:
                            pend.pop(0)[1]()
                    # reciprocals off SBUF copies of the colsum rows, then
                    # evict + normalize this chunk on DVE/GpSimd while the
                    # PE streams on
                    t_cs = csp.tile([1, 2, 512], F32, tag="cs", name=f"cs{t}{ch}")
                    nc.vector.tensor_copy(t_cs[0:1, 0, :], yt0[D:D1, :])
                    nc.vector.tensor_copy(t_cs[0:1, 1, :], yt1[D:D1, :])
                    t_rc = csp.tile([1, 2, 512], F32, tag="rc", name=f"rc{t}{ch}")
                    nc.vector.reciprocal_approx_fast(t_rc[0:1, 0, :],
                                                     t_cs[0:1, 0, :])
                    nc.vector.reciprocal_approx_fast(t_rc[0:1, 1, :],
                                                     t_cs[0:1, 1, :])
                    nc.vector.tensor_copy(t_yt[t][0:64, nsl], yt0[0:D, :])
                    nc.vector.tensor_copy(t_yt[t][64:128, nsl], yt1[0:D, :])
                    for hp in range(2):
                        psl = slice(hp * 64, hp * 64 + 64)
                        t_bc = bcp.tile([128, 512], F32, tag="bc",
                                        name=f"bc{t}{ch}{hp}")
                        nc.gpsimd.partition_broadcast(
                            t_bc[:], t_rc[0:1, hp, :]
                        )
                        nc.vector.tensor_mul(
                            t_yt[t][psl, nsl], t_yt[t][psl, nsl], t_bc[psl, :]
                        )
                    # anything still pending that this pair needs must land
                    # before the next chunk reads it
                    due = [pu for pu in pend if pu[0] <= t]
                    if due:
                        for i, u in due:
                            u()
                        pend = [pu for pu in pend if pu[0] > t]
                # next pair's projections must be complete before it starts
                for i, u in [pu for pu in pend if pu[0] == t + 1]:
                    u()
                pend = [pu for pu in pend if pu[0] != t + 1]
                if t + 1 < KO:
                    t_q, t_k = qk_tiles[t + 1]

            # ---- output projection: out[n, c] = Yt.T @ pwT; yt stationary
            # shared by both output chunks (halves the LDWEIGHTS). ch1
            # tokens (mt 4-7) first: the last pair normalizes them first.
            # Accumulators split over the st + mm pools so consecutive
            # token tiles overlap with the evictions. ----
            for mt in [4, 5, 6, 7, 0, 1, 2, 3]:
                t_o = obp.tile([128, C], MMD, tag="ob", name=f"ob{mt}")
                msl = slice(mt * 128, (mt + 1) * 128)
                psA = stp.tile([128, 512], F32, tag="st", name=f"poA{mt}")
                psB = mmp.tile([128, 512], F32, tag="mm", name=f"poB{mt}")
                for t in range(KO):
                    nc.tensor.matmul(
                        psA[:], t_yt[t][:, msl], t_wp[:, t, 0:512],
                        start=(t == 0), stop=(t == KO - 1),
                    )
                    nc.tensor.matmul(
                        psB[:, 0:256], t_yt[t][:, msl], t_wp[:, t, 512:768],
                        start=(t == 0), stop=(t == KO - 1),
                    )
                nc.vector.tensor_copy(t_o[:, 0:512], psA[:])
                nc.vector.tensor_copy(t_o[:, 512:768], psB[:, 0:256])
                nc.sync.dma_start(out=r_out[mt, :, :], in_=t_o[:])

    nc.compile()
    return nc


def _prep_inputs(x, head_mask, q_w, k_w, v_w, proj_w):
    import ml_dtypes

    mmnp = {"bf16": ml_dtypes.bfloat16, "f16": np.float16,
            "f32r": np.float32, "f32": np.float32}[MM_DTYPE]
    scale = np.float32(D ** -0.5)
    wqT = np.ascontiguousarray((q_w * scale).T).astype(mmnp)
    wkT = np.ascontiguousarray(k_w.T).astype(mmnp)
    vwT0 = np.zeros((C, CV), np.float32)
    vT = v_w.T.astype(np.float32)
    for h in range(H):
        vwT0[:, h * D1:h * D1 + D] = vT[:, h * D:(h + 1) * D]
    pwT = np.ascontiguousarray(proj_w.T).astype(mmnp)
    in_maps = []
    for b in range(NCORES):
        xT = np.ascontiguousarray(x[b].T).astype(mmnp)
        # fold head_mask^2 into this core's V weights (ones cols stay 0->1)
        vwT = vwT0.copy()
        for h in range(H):
            vwT[:, h * D1:h * D1 + D] *= head_mask[b, h] ** 2
        in_maps.append(
            {"xT": xT, "wqT": wqT, "wkT": wkT, "vwT": vwT.astype(mmnp),
             "pwT": pwT}
        )
    return in_maps


def _run(inputs, trace=False):
    from concourse.bass_utils import run_bass_kernel_spmd

    x = np.asarray(inputs["x"], np.float32)
    head_mask = np.asarray(inputs["head_mask"], np.float32)
    in_maps = _prep_inputs(
        x,
        head_mask,
        np.asarray(inputs["q_w"], np.float32),
        np.asarray(inputs["k_w"], np.float32),
        np.asarray(inputs["v_w"], np.float32),
        np.asarray(inputs["proj_w"], np.float32),
    )
    # biases are zero by construction of this problem (spec fill=zeros);
    # q_b/k_b/v_b/proj_b are validated and otherwise unused.
    for name in ("q_b", "k_b", "v_b", "proj_b"):
        bias = np.asarray(inputs[name])
        if np.abs(bias).max() > 0:
            raise NotImplementedError(f"nonzero {name} not supported")

    if "nc" not in _cache:
        _cache["nc"] = _build()
    nc = _cache["nc"]
    res = run_bass_kernel_spmd(
        nc, in_maps, core_ids=list(range(NCORES)), trace=trace
    )
    out = np.stack([res.results[b]["out"] for b in range(NCORES)], axis=0)
    return out.astype(np.float32), res


def kernel(**inputs):
    out, _ = _run(inputs, trace=False)
    return out
